# revision 7
# baseline (speedup 1.0000x reference)
"""GAT-style masked-softmax attention kernel for Trainium2 (8 NeuronCores).

Problem (per batch b of 32):
    e   = leaky_relu(h @ a1 + (g @ a2)^T, 0.2)        # (N, M)
    att = softmax(where(adj > 0, e, -9e15), axis=-1)  # (N, M)
    out = (att * adj.sum(-1, keepdims=True)) @ g      # (N, D)

Strategy (pure data parallel over B=32 -> 4 batches/core):
  * No row-max subtraction: e is bounded (~|e| <= 20), exp() is fp32-safe, and
    softmax is shift-invariant, so s = adj * exp(e) and out_i = (deg_i /
    rowsum_i) * (s @ g)_i.  All-masked rows (prob ~2^-1024) would give 0/0;
    ignored.
  * Scores are built TRANSPOSED (j on partitions) directly by the tensor
    engine as an outer sum (two K=1 matmuls), so s^T feeds the second matmul
    as lhsT with no on-chip transpose of s.
  * adj int32 is cast to bf16 on DVE (one pass, fused row-degree via
    accum_out), then transposed by the DMA xbar (2-byte path).
  * leaky_relu = ACT Prelu(alpha=0.2); exp = ACT Exp. Both live in the
    "exp_and_others" table set -> no table reloads.
  * rowsum(s) rides the second matmul as a ones-column appended to g.
  * final scale deg/rowsum applied per-partition by DVE tensor_scalar.

Self-contained: hardcodes shapes B,N,M,D = 32,1024,1024,128 on 8 cores.
"""

import sys

if "/opt/trn_rl_repo" not in sys.path:
    sys.path.insert(0, "/opt/trn_rl_repo")

import numpy as np

import concourse.bacc as bacc
import concourse.mybir as mybir
import concourse.tile as tile
import concourse.bass_utils as bass_utils

F32 = mybir.dt.float32
BF16 = mybir.dt.bfloat16
I32 = mybir.dt.int32
OP = mybir.AluOpType
AF = mybir.ActivationFunctionType

B, N, M, D = 32, 1024, 1024, 128
NCORES = 8
BPC = B // NCORES  # batches per core
NI = N // 128      # i blocks
NJ = M // 128      # j blocks


def build_bass():
    nc = bacc.Bacc("TRN2", target_bir_lowering=False, debug=False)

    h_in = nc.dram_tensor("input1", [BPC, N, D], F32, kind="ExternalInput").ap()
    g_in = nc.dram_tensor("input2", [BPC, M, D], F32, kind="ExternalInput").ap()
    adj_in = nc.dram_tensor("adj", [BPC, N, M], I32, kind="ExternalInput").ap()
    a1_in = nc.dram_tensor("a1", [D, 1], F32, kind="ExternalInput").ap()
    a2_in = nc.dram_tensor("a2", [D, 1], F32, kind="ExternalInput").ap()
    out_d = nc.dram_tensor("out", [BPC, N, D], F32, kind="ExternalOutput").ap()

    urow_scr = nc.dram_tensor("urow_scr", [BPC, N], F32).ap()
    vrow_scr = nc.dram_tensor("vrow_scr", [BPC, M], F32).ap()
    deg_scr = nc.dram_tensor("deg_scr", [BPC, N], F32).ap()

    with tile.TileContext(nc) as tc:
        with (
            tc.tile_pool(name="singles", bufs=1) as singles,
            tc.tile_pool(name="hg", bufs=2) as hg_pool,
            tc.tile_pool(name="gbf", bufs=2) as gbf_pool,
            tc.tile_pool(name="adjp", bufs=3) as adj_pool,
            tc.tile_pool(name="afp", bufs=3) as af_pool,
            tc.tile_pool(name="aft", bufs=2) as aft_pool,
            tc.tile_pool(name="plp", bufs=3) as pl_pool,
            tc.tile_pool(name="exp", bufs=3) as ex_pool,
            tc.tile_pool(name="st", bufs=2) as st_pool,
            tc.tile_pool(name="small", bufs=4) as small,
            tc.tile_pool(name="outp", bufs=4) as out_pool,
            tc.tile_pool(name="psE", bufs=2, space="PSUM") as psE,
            tc.tile_pool(name="psO", bufs=2, space="PSUM") as psO,
            tc.tile_pool(name="psD", bufs=1, space="PSUM") as psD,
        ):
            # ---- static prep -------------------------------------------------
            ones_row = singles.tile([1, max(N, M)], F32)
            nc.vector.memset(ones_row[:], 1.0)
            ones_sq_bf = singles.tile([128, 128], BF16)
            nc.vector.memset(ones_sq_bf[:], 1.0)

            a1row = singles.tile([1, D], F32)
            nc.sync.dma_start(a1row[:], a1_in.transpose((1, 0)))
            a2row = singles.tile([1, D], F32)
            nc.sync.dma_start(a2row[:], a2_in.transpose((1, 0)))

            a1bc = singles.tile([128, D], F32)
            a2bc = singles.tile([128, D], F32)
            bc_ps = psO.tile([128, D], F32, tag="o_ps")
            nc.tensor.matmul(bc_ps[:], ones_row[:, :128], a1row[:], start=True, stop=True)
            nc.vector.tensor_copy(a1bc[:], bc_ps[:])
            bc_ps2 = psO.tile([128, D], F32, tag="o_ps")
            nc.tensor.matmul(bc_ps2[:], ones_row[:, :128], a2row[:], start=True, stop=True)
            nc.vector.tensor_copy(a2bc[:], bc_ps2[:])

            for b in range(BPC):
                # ---- h/g loads ----------------------------------------------
                h_t = hg_pool.tile([128, NI, D], F32, tag="h")
                nc.sync.dma_start(
                    h_t[:], h_in[b].rearrange("(ib p) d -> p ib d", p=128)
                )
                g_t = hg_pool.tile([128, NJ, D], F32, tag="g")
                nc.sync.dma_start(
                    g_t[:], g_in[b].rearrange("(jb p) d -> p jb d", p=128)
                )
                # bf16 copy of g with ones column for rowsum
                g_bf = gbf_pool.tile([128, NJ, D + 2], BF16)
                nc.vector.memset(g_bf[:, :, D : D + 1], 1.0)
                nc.vector.memset(g_bf[:, :, D + 1 : D + 2], 0.0)
                nc.vector.tensor_copy(g_bf[:, :, 0:D], g_t[:])

                # ---- u/v projections ----------------------------------------
                ucols = small.tile([128, NI], F32, tag="ucols")
                vcols = small.tile([128, NJ], F32, tag="vcols")
                uscr = small.tile([128, D], F32, tag="uscr")
                for ib in range(NI):
                    nc.vector.scalar_tensor_tensor(
                        uscr[:], h_t[:, ib, :], 0.0, a1bc[:],
                        OP.bypass, OP.mult, accum_out=ucols[:, ib : ib + 1],
                    )
                for jb in range(NJ):
                    nc.vector.scalar_tensor_tensor(
                        uscr[:], g_t[:, jb, :], 0.0, a2bc[:],
                        OP.bypass, OP.mult, accum_out=vcols[:, jb : jb + 1],
                    )
                # cols -> rows via DRAM bounce
                nc.sync.dma_start(
                    urow_scr[b].rearrange("(ib p) -> p ib", p=128), ucols[:]
                )
                nc.sync.dma_start(
                    vrow_scr[b].rearrange("(jb p) -> p jb", p=128), vcols[:]
                )
                urow = small.tile([1, N], F32, tag="urow")
                nc.sync.dma_start(urow[:], urow_scr[b].unsqueeze(0))
                vrow = small.tile([1, M], F32, tag="vrow")
                nc.sync.dma_start(vrow[:], vrow_scr[b].unsqueeze(0))

                # ---- adj: load, cast, xbar transpose ------------------------
                afT = aft_pool.tile([128, NJ, N], BF16)
                for ib in range(NI):
                    adj_t = adj_pool.tile([128, M], I32)
                    nc.sync.dma_start(adj_t[:], adj_in[b, ib * 128 : (ib + 1) * 128, :])
                    af = af_pool.tile([128, M], BF16)
                    nc.vector.tensor_scalar(af[:], adj_t[:], 1.0, None, OP.mult)
                    # afT[p, jb, i] = af^T[jb*128+p, i] for i in this ib block
                    nc.sync.dma_start_transpose(
                        afT[:, :, ib * 128 : (ib + 1) * 128], af[:]
                    )

                # ---- deg_i = sum_j adj[i, j] via PE ones-matmul -------------
                # replicated-row form, then bounce through DRAM to column form
                deg_ps = psD.tile([128, N], F32)
                for jb in range(NJ):
                    for half in range(2):
                        fs = slice(half * 512, (half + 1) * 512)
                        nc.tensor.matmul(
                            deg_ps[:, fs], ones_sq_bf[:], afT[:, jb, fs],
                            start=(jb == 0), stop=(jb == NJ - 1),
                        )
                degrow = small.tile([128, N], F32, tag="degrow")
                nc.vector.tensor_copy(degrow[:], deg_ps[:])
                nc.sync.dma_start(deg_scr[b].unsqueeze(0), degrow[0:1, :])
                deg = small.tile([128, NI], F32, tag="deg")
                nc.sync.dma_start(
                    deg[:], deg_scr[b].rearrange("(ib p) -> p ib", p=128)
                )

                # ---- E^T -> prelu -> exp -> mask (per jb) -------------------
                sT = st_pool.tile([128, NJ, N], BF16)
                pl = None
                for jb in range(NJ):
                    e_ps = psE.tile([128, N], F32)
                    vsl = vrow[0:1, jb * 128 : (jb + 1) * 128]
                    for half in range(2):
                        fs = slice(half * 512, (half + 1) * 512)
                        nc.tensor.matmul(
                            e_ps[:, fs], vsl, ones_row[0:1, 0:512],
                            start=True, stop=False,
                        )
                        nc.tensor.matmul(
                            e_ps[:, fs], ones_row[0:1, 0:128], urow[0:1, fs],
                            start=False, stop=True,
                        )
                    if jb % 2 == 0:
                        pl = pl_pool.tile([128, 2, N], F32)
                    nc.scalar.activation(
                        pl[:, jb % 2, :], e_ps[:], AF.Prelu, alpha=0.2
                    )
                    if jb % 2 == 1:
                        ex = ex_pool.tile([128, 2, N], BF16)
                        nc.scalar.activation(ex[:], pl[:], AF.Exp)
                        for j2 in (jb - 1, jb):
                            nc.vector.tensor_tensor(
                                sT[:, j2, :], afT[:, j2, :], ex[:, j2 % 2, :],
                                OP.mult,
                            )

                # ---- main matmul + epilogue (per ib) ------------------------
                for ib in range(NI):
                    o_ps = psO.tile([128, D + 2], F32, tag="o_ps")
                    for jb in range(NJ):
                        nc.tensor.matmul(
                            o_ps[:],
                            sT[:, jb, ib * 128 : (ib + 1) * 128],
                            g_bf[:, jb, :],
                            start=(jb == 0),
                            stop=(jb == NJ - 1),
                        )
                    rs = small.tile([128, 1], F32, tag="rs")
                    nc.vector.reciprocal(rs[:], o_ps[:, D : D + 1])
                    fac = small.tile([128, 1], F32, tag="fac")
                    nc.vector.tensor_tensor(fac[:], rs[:], deg[:, ib : ib + 1], OP.mult)
                    out_sb = out_pool.tile([128, D], F32)
                    nc.vector.tensor_scalar(
                        out_sb[:], o_ps[:, 0:D], fac[:], None, OP.mult
                    )
                    nc.sync.dma_start(
                        out_d[b, ib * 128 : (ib + 1) * 128, :], out_sb[:]
                    )

    nc.compile()
    return nc


_CACHE = {}


def _get_nc():
    if "nc" not in _CACHE:
        _CACHE["nc"] = build_bass()
    return _CACHE["nc"]


def kernel(input1, input2, adj, a1, a2):
    nc = _get_nc()
    input1 = np.ascontiguousarray(np.asarray(input1, dtype=np.float32))
    input2 = np.ascontiguousarray(np.asarray(input2, dtype=np.float32))
    adj = np.ascontiguousarray(np.asarray(adj, dtype=np.int32))
    a1 = np.ascontiguousarray(np.asarray(a1, dtype=np.float32))
    a2 = np.ascontiguousarray(np.asarray(a2, dtype=np.float32))

    in_maps = []
    for c in range(NCORES):
        sl = slice(c * BPC, (c + 1) * BPC)
        in_maps.append(
            {
                "input1": input1[sl],
                "input2": input2[sl],
                "adj": adj[sl],
                "a1": a1,
                "a2": a2,
            }
        )
    res = bass_utils.run_bass_kernel_spmd(nc, in_maps, core_ids=list(range(NCORES)))
    return np.concatenate([r["out"] for r in res.results], axis=0)


def run_traced(input1, input2, adj, a1, a2, trace_cores=None):
    """Like kernel() but returns (output, BassKernelResults) with trace=True."""
    nc = _get_nc()
    input1 = np.ascontiguousarray(np.asarray(input1, dtype=np.float32))
    input2 = np.ascontiguousarray(np.asarray(input2, dtype=np.float32))
    adj = np.ascontiguousarray(np.asarray(adj, dtype=np.int32))
    a1 = np.ascontiguousarray(np.asarray(a1, dtype=np.float32))
    a2 = np.ascontiguousarray(np.asarray(a2, dtype=np.float32))
    in_maps = []
    for c in range(NCORES):
        sl = slice(c * BPC, (c + 1) * BPC)
        in_maps.append(
            {
                "input1": input1[sl],
                "input2": input2[sl],
                "adj": adj[sl],
                "a1": a1,
                "a2": a2,
            }
        )
    res = bass_utils.run_bass_kernel_spmd(
        nc,
        in_maps,
        core_ids=list(range(NCORES)),
        trace=True,
        trace_cores=trace_cores or [0],
    )
    out = np.concatenate([r["out"] for r in res.results], axis=0)
    return out, res


# revision 10
# speedup vs baseline: 1.2429x; 1.2429x over previous
"""GAT-style masked-softmax attention kernel for Trainium2 (8 NeuronCores).

Problem (per batch b of 32):
    e   = leaky_relu(h @ a1 + (g @ a2)^T, 0.2)        # (N, M)
    att = softmax(where(adj > 0, e, -9e15), axis=-1)  # (N, M)
    out = (att * adj.sum(-1, keepdims=True)) @ g      # (N, D)

Strategy (pure data parallel over B=32 -> 4 batches/core):
  * No row-max subtraction: e is bounded (~|e| <= 20), exp() is fp32-safe, and
    softmax is shift-invariant, so s = adj * exp(e) and out_i = (deg_i /
    rowsum_i) * (s @ g)_i.  All-masked rows (prob ~2^-1024) would give 0/0;
    ignored.
  * Scores are built TRANSPOSED (j on partitions) directly by the tensor
    engine as an outer sum (two K=1 matmuls), so s^T feeds the second matmul
    as lhsT with no on-chip transpose of s.
  * adj int32 is cast to bf16 on DVE (one pass, fused row-degree via
    accum_out), then transposed by the DMA xbar (2-byte path).
  * leaky_relu = ACT Prelu(alpha=0.2); exp = ACT Exp. Both live in the
    "exp_and_others" table set -> no table reloads.
  * rowsum(s) rides the second matmul as a ones-column appended to g.
  * final scale deg/rowsum applied per-partition by DVE tensor_scalar.

Self-contained: hardcodes shapes B,N,M,D = 32,1024,1024,128 on 8 cores.
"""

import sys

if "/opt/trn_rl_repo" not in sys.path:
    sys.path.insert(0, "/opt/trn_rl_repo")

import numpy as np

import concourse.bacc as bacc
import concourse.mybir as mybir
import concourse.tile as tile
import concourse.bass_utils as bass_utils

F32 = mybir.dt.float32
BF16 = mybir.dt.bfloat16
I32 = mybir.dt.int32
OP = mybir.AluOpType
AF = mybir.ActivationFunctionType

B, N, M, D = 32, 1024, 1024, 128
NCORES = 8
BPC = B // NCORES  # batches per core
NI = N // 128      # i blocks
NJ = M // 128      # j blocks


def build_bass():
    nc = bacc.Bacc("TRN2", target_bir_lowering=False, debug=False)

    h_in = nc.dram_tensor("input1", [BPC, N, D], F32, kind="ExternalInput").ap()
    g_in = nc.dram_tensor("input2", [BPC, M, D], F32, kind="ExternalInput").ap()
    adj_in = nc.dram_tensor("adj", [BPC, N, M], I32, kind="ExternalInput").ap()
    a1_in = nc.dram_tensor("a1", [D, 1], F32, kind="ExternalInput").ap()
    a2_in = nc.dram_tensor("a2", [D, 1], F32, kind="ExternalInput").ap()
    out_d = nc.dram_tensor("out", [BPC, N, D], F32, kind="ExternalOutput").ap()

    urow_scr = nc.dram_tensor("urow_scr", [BPC, N], F32).ap()
    vrow_scr = nc.dram_tensor("vrow_scr", [BPC, M], F32).ap()
    deg_scr = nc.dram_tensor("deg_scr", [BPC, N], F32).ap()

    with tile.TileContext(nc) as tc:
        with (
            tc.tile_pool(name="singles", bufs=1) as singles,
            tc.tile_pool(name="hg", bufs=2) as hg_pool,
            tc.tile_pool(name="gbf", bufs=2) as gbf_pool,
            tc.tile_pool(name="adjp", bufs=3) as adj_pool,
            tc.tile_pool(name="afp", bufs=3) as af_pool,
            tc.tile_pool(name="aft", bufs=2) as aft_pool,
            tc.tile_pool(name="plp", bufs=2) as pl_pool,
            tc.tile_pool(name="exp", bufs=2) as ex_pool,
            tc.tile_pool(name="st", bufs=2) as st_pool,
            tc.tile_pool(name="small", bufs=4) as small,
            tc.tile_pool(name="rows", bufs=2) as rows_pool,
            tc.tile_pool(name="outp", bufs=4) as out_pool,
            tc.tile_pool(name="psU", bufs=2, space="PSUM") as psU,
            tc.tile_pool(name="psO", bufs=2, space="PSUM") as psO,
            tc.tile_pool(name="psD", bufs=1, space="PSUM") as psD,
        ):
            # ---- static prep -------------------------------------------------
            ones_row = singles.tile([1, max(N, M)], F32)
            nc.vector.memset(ones_row[:], 1.0)
            ones_sq_bf = singles.tile([128, 128], BF16)
            nc.vector.memset(ones_sq_bf[:], 1.0)

            a1row = singles.tile([1, D], F32)
            nc.sync.dma_start(a1row[:], a1_in.transpose((1, 0)))
            a2row = singles.tile([1, D], F32)
            nc.sync.dma_start(a2row[:], a2_in.transpose((1, 0)))

            a1bc = singles.tile([128, D], F32)
            a2bc = singles.tile([128, D], F32)
            bc_ps = psO.tile([128, D], F32, tag="o_ps")
            nc.tensor.matmul(bc_ps[:], ones_row[:, :128], a1row[:], start=True, stop=True)
            nc.vector.tensor_copy(a1bc[:], bc_ps[:])
            bc_ps2 = psO.tile([128, D], F32, tag="o_ps")
            nc.tensor.matmul(bc_ps2[:], ones_row[:, :128], a2row[:], start=True, stop=True)
            nc.vector.tensor_copy(a2bc[:], bc_ps2[:])

            for b in range(BPC):
                # ---- h/g loads ----------------------------------------------
                h_t = hg_pool.tile([128, NI, D], F32, tag="h")
                nc.sync.dma_start(
                    h_t[:], h_in[b].rearrange("(ib p) d -> p ib d", p=128)
                )
                g_t = hg_pool.tile([128, NJ, D], F32, tag="g")
                nc.sync.dma_start(
                    g_t[:], g_in[b].rearrange("(jb p) d -> p jb d", p=128)
                )
                # bf16 copy of g with ones column for rowsum
                g_bf = gbf_pool.tile([128, NJ, D + 2], BF16)
                nc.vector.memset(g_bf[:, :, D : D + 1], 1.0)
                nc.vector.memset(g_bf[:, :, D + 1 : D + 2], 0.0)
                nc.vector.tensor_copy(g_bf[:, :, 0:D], g_t[:])

                # ---- u/v projections ----------------------------------------
                ucols = small.tile([128, NI], F32, tag="ucols")
                vcols = small.tile([128, NJ], F32, tag="vcols")
                uscr = small.tile([128, D], F32, tag="uscr")
                for ib in range(NI):
                    nc.vector.scalar_tensor_tensor(
                        uscr[:], h_t[:, ib, :], 0.0, a1bc[:],
                        OP.bypass, OP.mult, accum_out=ucols[:, ib : ib + 1],
                    )
                for jb in range(NJ):
                    nc.vector.scalar_tensor_tensor(
                        uscr[:], g_t[:, jb, :], 0.0, a2bc[:],
                        OP.bypass, OP.mult, accum_out=vcols[:, jb : jb + 1],
                    )
                # u cols -> row via DRAM bounce (v stays as columns: it is
                # consumed as a per-partition activation bias)
                nc.sync.dma_start(
                    urow_scr[b].rearrange("(ib p) -> p ib", p=128), ucols[:]
                )
                urow = rows_pool.tile([1, N], F32, tag="urow")
                nc.sync.dma_start(urow[:], urow_scr[b].unsqueeze(0))

                # ---- adj: load, cast, xbar transpose ------------------------
                # afT[p, ib, jb, i'] = adj[b, ib*128+i', jb*128+p]
                # (per-call dest afT[:, ib] is contiguous -> fast xbar writes)
                afT = aft_pool.tile([128, NI, NJ, 128], BF16)
                for ib in range(NI):
                    adj_t = adj_pool.tile([128, M], I32)
                    nc.sync.dma_start(adj_t[:], adj_in[b, ib * 128 : (ib + 1) * 128, :])
                    af = af_pool.tile([128, M], BF16)
                    nc.vector.tensor_scalar(af[:], adj_t[:], 1.0, None, OP.mult)
                    nc.sync.dma_start_transpose(afT[:, ib], af[:])

                # ---- deg_i = sum_j adj[i, j] via PE ones-matmul -------------
                # replicated-row form, then bounce through DRAM to column form
                deg_ps = psD.tile([128, N], F32)
                for jb in range(NJ):
                    for half in range(2):
                        ibs = slice(half * 4, (half + 1) * 4)
                        fs = slice(half * 512, (half + 1) * 512)
                        nc.tensor.matmul(
                            deg_ps[:, fs], ones_sq_bf[:], afT[:, ibs, jb, :],
                            start=(jb == 0), stop=(jb == NJ - 1),
                        )
                degrow = rows_pool.tile([128, N], F32, tag="degrow")
                nc.vector.tensor_copy(degrow[:], deg_ps[:])
                nc.sync.dma_start(deg_scr[b].unsqueeze(0), degrow[0:1, :])
                deg = small.tile([128, NI], F32, tag="deg")
                nc.sync.dma_start(
                    deg[:], deg_scr[b].rearrange("(ib p) -> p ib", p=128)
                )

                # ---- u broadcast, then prelu(u_i + v_j) -> exp -> mask ------
                # u_repl[p, i] = u_i via K=1 ones matmul; v_j enters as the
                # per-partition activation bias (j is the partition dim here).
                u_ps = psU.tile([128, N], F32)
                for half in range(2):
                    fs = slice(half * 512, (half + 1) * 512)
                    nc.tensor.matmul(
                        u_ps[:, fs], ones_row[0:1, 0:128], urow[0:1, fs],
                        start=True, stop=True,
                    )
                sT = st_pool.tile([128, NJ, N], BF16)
                pl = None
                for jb in range(NJ):
                    if jb % 4 == 0:
                        pl = pl_pool.tile([128, 4, N], F32)
                    nc.scalar.activation(
                        pl[:, jb % 4, :], u_ps[:], AF.Prelu,
                        bias=vcols[:, jb : jb + 1], alpha=0.2,
                    )
                    if jb % 4 == 3:
                        ex = ex_pool.tile([128, 4, N], BF16)
                        nc.scalar.activation(ex[:], pl[:], AF.Exp)
                        for j2 in (jb - 3, jb - 1):
                            nc.vector.tensor_tensor(
                                sT[:, j2 : j2 + 2, :].rearrange(
                                    "p a (ib c) -> p a ib c", ib=NI
                                ),
                                afT[:, :, j2 : j2 + 2, :].transpose((0, 2, 1, 3)),
                                ex[:, j2 % 4 : j2 % 4 + 2, :].rearrange(
                                    "p a (ib c) -> p a ib c", ib=NI
                                ),
                                OP.mult,
                            )

                # ---- main matmul + epilogue (per ib) ------------------------
                for ib in range(NI):
                    o_ps = psO.tile([128, D + 2], F32, tag="o_ps")
                    for jb in range(NJ):
                        nc.tensor.matmul(
                            o_ps[:],
                            sT[:, jb, ib * 128 : (ib + 1) * 128],
                            g_bf[:, jb, :],
                            start=(jb == 0),
                            stop=(jb == NJ - 1),
                        )
                    rs = small.tile([128, 1], F32, tag="rs")
                    nc.vector.reciprocal(rs[:], o_ps[:, D : D + 1])
                    fac = small.tile([128, 1], F32, tag="fac")
                    nc.vector.tensor_tensor(fac[:], rs[:], deg[:, ib : ib + 1], OP.mult)
                    out_sb = out_pool.tile([128, D], F32)
                    nc.vector.tensor_scalar(
                        out_sb[:], o_ps[:, 0:D], fac[:], None, OP.mult
                    )
                    nc.sync.dma_start(
                        out_d[b, ib * 128 : (ib + 1) * 128, :], out_sb[:]
                    )

    nc.compile()
    return nc


_CACHE = {}


def _get_nc():
    if "nc" not in _CACHE:
        _CACHE["nc"] = build_bass()
    return _CACHE["nc"]


def kernel(input1, input2, adj, a1, a2):
    nc = _get_nc()
    input1 = np.ascontiguousarray(np.asarray(input1, dtype=np.float32))
    input2 = np.ascontiguousarray(np.asarray(input2, dtype=np.float32))
    adj = np.ascontiguousarray(np.asarray(adj, dtype=np.int32))
    a1 = np.ascontiguousarray(np.asarray(a1, dtype=np.float32))
    a2 = np.ascontiguousarray(np.asarray(a2, dtype=np.float32))

    in_maps = []
    for c in range(NCORES):
        sl = slice(c * BPC, (c + 1) * BPC)
        in_maps.append(
            {
                "input1": input1[sl],
                "input2": input2[sl],
                "adj": adj[sl],
                "a1": a1,
                "a2": a2,
            }
        )
    res = bass_utils.run_bass_kernel_spmd(nc, in_maps, core_ids=list(range(NCORES)))
    return np.concatenate([r["out"] for r in res.results], axis=0)


def run_traced(input1, input2, adj, a1, a2, trace_cores=None):
    """Like kernel() but returns (output, BassKernelResults) with trace=True."""
    nc = _get_nc()
    input1 = np.ascontiguousarray(np.asarray(input1, dtype=np.float32))
    input2 = np.ascontiguousarray(np.asarray(input2, dtype=np.float32))
    adj = np.ascontiguousarray(np.asarray(adj, dtype=np.int32))
    a1 = np.ascontiguousarray(np.asarray(a1, dtype=np.float32))
    a2 = np.ascontiguousarray(np.asarray(a2, dtype=np.float32))
    in_maps = []
    for c in range(NCORES):
        sl = slice(c * BPC, (c + 1) * BPC)
        in_maps.append(
            {
                "input1": input1[sl],
                "input2": input2[sl],
                "adj": adj[sl],
                "a1": a1,
                "a2": a2,
            }
        )
    res = bass_utils.run_bass_kernel_spmd(
        nc,
        in_maps,
        core_ids=list(range(NCORES)),
        trace=True,
        trace_cores=trace_cores or [0],
    )
    out = np.concatenate([r["out"] for r in res.results], axis=0)
    return out, res
